# revision 13
# baseline (speedup 1.0000x reference)
"""TRN2 Bass kernel for nn_ChartOperator (sparse_attention), v4.

Math (B=4, N=4096, PD=1024, D=16, S=64, ALL=1024):
  P = x @ W_r + b_r
  L = P[..., :ALL].reshape(n, D, S); R = P[..., ALL:].reshape(n, D, S)
  e = exp(L); Z = sum_a e[n, a, s]
  Q[n, d, s] = (1/Z[n,s]) * sum_{a<d} e[n,a,s] * R[n+a+1, d-1-a, s]
  (last D rows of each batch: Q[t+d>=16] zeroed)
  out = Q.reshape(n, ALL) @ W_w + b_w

Sharding: 8 cores data-parallel over flattened (B*N) rows, 2048 rows/core.
The 16-row forward halo R is computed on the HOST (16x1024x1024 matmul per
core) and DMA'd straight into the rt tail; odd cores get bias-only halo
(their affected outputs are exactly the masked ones).

Two row-chains packed in partitions: chain g0 = rows 0..1023, g1 = rows
1024..2047, partition = (g, s). Each reader supertile covers 256 rows of
BOTH chains (rows [256i, 256i+256) u [1024+256i, ...)), so every supertile
unlocks a full 256-col MAC window across both chains and the DVE MAC
streams continuously behind the reader instead of stalling on the tail.

Engine budget (per core): PE = reader 112.6us + writer 54.6us (matmul only,
banded MAC is NOT on PE). DVE = products + adds (bf16 2x; misaligned bf16
still runs 2x, measured) + tree-Z + qt(rz-folded) ~ 165us. ACT = exp/ident
PSUM drains + osb. GpSimd: DMA issue only - concurrent GpSimd tensor ops
stall DVE on SBUF ports (measured: full op-duration stalls).
"""
import numpy as np
import ml_dtypes
from contextlib import ExitStack

import concourse.bass as bass
import concourse.tile as tile
from concourse import bacc, mybir
from concourse import bass_utils

BF16 = mybir.dt.bfloat16
F32 = mybir.dt.float32
bfnp = ml_dtypes.bfloat16
AF = mybir.ActivationFunctionType

B, N, PD = 4, 4096, 1024
D, S = 16, 64
ALL = D * S
ROWS_PER_CORE = 2048
RT_LEN = 1040                  # rt chain length (max read idx 1038)

# MAC windows (n0, W): read limit n0+W-1+15 must stay within the cols
# unlocked by supertiles 0..i (256*(i+1)-1) resp. 1039 for the last.
SCS = [(0, 240), (240, 256), (496, 256), (752, 256), (1008, 16)]
# cb -> list of (window idx, src_off, dst_off, width)
CB_SRC = {
    0: [(0, 0, 0, 128)],
    1: [(0, 128, 0, 112), (1, 0, 112, 16)],
    2: [(1, 16, 0, 128)],
    3: [(1, 144, 0, 112), (2, 0, 112, 16)],
    4: [(2, 16, 0, 128)],
    5: [(2, 144, 0, 112), (3, 0, 112, 16)],
    6: [(3, 16, 0, 128)],
    7: [(3, 144, 0, 112), (4, 0, 112, 16)],
}

_cache = {}


def _build(debug=False):
    nc = bacc.Bacc("TRN2", target_bir_lowering=False, debug=False, num_devices=8)

    x4_d = nc.dram_tensor("x4", [4, 8, 128, 512], BF16, kind="ExternalInput").ap()
    rh_d = nc.dram_tensor("rh", [64, 15, 16], BF16, kind="ExternalInput").ap()
    wr_d = nc.dram_tensor("wr", [8, 128, 2048], BF16, kind="ExternalInput").ap()
    ww_d = nc.dram_tensor("ww", [8, 128, 1024], BF16, kind="ExternalInput").ap()
    br_d = nc.dram_tensor("br", [128, 16], F32, kind="ExternalInput").ap()
    qmask_d = nc.dram_tensor("qmask", [128, 8, 128], BF16, kind="ExternalInput").ap()
    out_d = nc.dram_tensor("out", [16, 128, 1024], BF16, kind="ExternalOutput").ap()

    with tile.TileContext(nc) as tc, ExitStack() as ctx, \
            nc.allow_low_precision(
                reason="bf16 MAC/tree validated vs reference: rel 7.3e-3 "
                       "< 2e-2 tolerance (numpy bit-model)"):
        cpool = ctx.enter_context(tc.tile_pool(name="cpool", bufs=1))
        xpool = ctx.enter_context(tc.tile_pool(name="xpool", bufs=3))
        treep = ctx.enter_context(tc.tile_pool(name="treep", bufs=2))
        prodp = ctx.enter_context(tc.tile_pool(name="prodp", bufs=3))
        accp = ctx.enter_context(tc.tile_pool(name="accp", bufs=2))
        qop = ctx.enter_context(tc.tile_pool(name="qop", bufs=4))
        psp = ctx.enter_context(tc.tile_pool(name="psp", bufs=3, space="PSUM"))
        wps = ctx.enter_context(tc.tile_pool(name="wps", bufs=2, space="PSUM"))

        # --- persistent constants / big buffers
        wr_sb = cpool.tile([128, 8, 2048], BF16)
        ww_sb = cpool.tile([128, 8, 1024], BF16)
        br_sb = cpool.tile([128, 16], F32)
        qmask = cpool.tile([128, 8, 128], BF16)
        et = cpool.tile([128, 16, 1024], BF16)    # [(g2,s64), a, n-chain] raw e
        rt = cpool.tile([128, 15, RT_LEN], BF16)  # [(g2,s64), c, n-chain] R
        rz = cpool.tile([128, 1024], BF16)        # 1/Z per chain col

        # weights on sync queue (first ks-pair first)
        for i in range(4):
            nc.sync.dma_start(wr_sb[:, 2 * i:2 * i + 2, :],
                              wr_d[2 * i:2 * i + 2].rearrange("k p c -> p k c"))
        nc.gpsimd.dma_start(br_sb[:], br_d[:])
        for i in range(2):
            nc.sync.dma_start(ww_sb[:, 4 * i:4 * i + 4, :],
                              ww_d[4 * i:4 * i + 4].rearrange("k p c -> p k c"))
        nc.gpsimd.dma_start(qmask[:], qmask_d[:])
        # host-computed halo R -> rt g1 tail
        nc.gpsimd.dma_start(rt[64:128, :, 1024:1040], rh_d[:])

        def load_x(i):
            xt = xpool.tile([128, 8, 512], BF16, tag="xk", name=f"x{i}")
            nc.gpsimd.dma_start(xt[:], x4_d[i].rearrange("k p n -> p k n"))
            return xt

        def reader_supertile(i, xt):
            # supertile i: rows [256i, 256i+256) = g0 cols [256i..] in ps
            # cols 0-255, rows [1024+256i, ..) = g1 cols in ps cols 256-511.
            n0 = 256 * i
            for u in range(16):           # col slabs: 0-7 = L, 8-15 = R
                is_l = u < 8
                ps = psp.tile([128, 512], F32, tag="ps", name="ps")
                for ks in range(8):
                    nc.tensor.matmul(ps[:], wr_sb[:, ks, 128 * u:128 * (u + 1)],
                                     xt[:, ks, :],
                                     start=(ks == 0), stop=(ks == 7))
                for dsub in range(2):
                    bias = br_sb[64 * dsub:64 * dsub + 64, u:u + 1]
                    for g in range(2):
                        src = ps[64 * dsub:64 * dsub + 64,
                                 256 * g:256 * g + 256]
                        if is_l:
                            a = 2 * u + dsub
                            nc.scalar.activation(
                                et[64 * g:64 * g + 64, a, n0:n0 + 256], src,
                                AF.Exp, bias=bias)
                        else:
                            c = 2 * (u - 8) + dsub
                            if c == 15:
                                continue  # c=15 never read by the MAC
                            nc.scalar.activation(
                                rt[64 * g:64 * g + 64, c, n0:n0 + 256], src,
                                AF.Identity, bias=bias)
                            if i == 0 and g == 1:
                                # rows 1024-1039 are ALSO g0 chain cols 1024+
                                nc.scalar.activation(
                                    rt[0:64, c, 1024:1040],
                                    ps[64 * dsub:64 * dsub + 64, 256:272],
                                    AF.Identity, bias=bias)

        def tree_z(w0, W):
            # rz = 1 / sum_a e over the 16 a-planes for chain cols [w0, w0+W)
            t1 = treep.tile([128, 8, 256], BF16, tag="tree", name="t1")
            nc.vector.tensor_add(t1[:, :, :W], et[:, 0:16:2, w0:w0 + W],
                                 et[:, 1:16:2, w0:w0 + W])
            t2 = treep.tile([128, 4, 256], BF16, tag="tree", name="t2")
            nc.vector.tensor_add(t2[:, :, :W], t1[:, 0:8:2, :W], t1[:, 1:8:2, :W])
            t3 = treep.tile([128, 2, 256], BF16, tag="tree", name="t3")
            nc.vector.tensor_add(t3[:, :, :W], t2[:, 0:4:2, :W], t2[:, 1:4:2, :W])
            zf = treep.tile([128, 256], F32, tag="tree", name="zf")
            nc.vector.tensor_add(zf[:, :W], t3[:, 0, :W], t3[:, 1, :W])
            nc.vector.reciprocal(rz[:, w0:w0 + W], zf[:, :W])
            # normalize e in place (w = e/Z): qt drains become plain ACT
            # copies, moving that work off the critical DVE tail
            rzb = rz[:, w0:w0 + W].rearrange("p (o n) -> p o n", o=1) \
                .to_broadcast((128, 16, W))
            nc.vector.tensor_mul(et[:, :, w0:w0 + W],
                                 et[:, :, w0:w0 + W], rzb)

        def mac_window(n0, W):
            # acc plane i (=d-1) over cols [n0, n0+W):
            #   acc[:, a+c, :] += e[:, a, n] * rt[:, c, n+a+1]
            # descending a: plane a is init'ed by its c=0 term. All on DVE:
            # concurrent GpSimd tensor ops stall DVE on SBUF ports, and
            # misaligned bf16 still runs 2x on DVE.
            acc = accp.tile([128, 15, 256], BF16, tag="acc", name="acc")
            for a in range(14, -1, -1):
                cnt = 15 - a
                sh = a + 1
                def eb(k):
                    return et[:, a:a + 1, n0:n0 + W].to_broadcast((128, k, W))
                nc.vector.tensor_mul(acc[:, a:a + 1, 0:W], eb(1),
                                     rt[:, 0:1, n0 + sh:n0 + sh + W])
                if cnt > 1:
                    k = cnt - 1
                    p = prodp.tile([128, 14, 256], BF16, tag="p", name="p")
                    nc.vector.tensor_mul(p[:, 0:k, 0:W], eb(k),
                                         rt[:, 1:cnt, n0 + sh:n0 + sh + W])
                    nc.vector.tensor_add(acc[:, a + 1:a + 1 + k, 0:W],
                                         acc[:, a + 1:a + 1 + k, 0:W],
                                         p[:, 0:k, 0:W])
            return acc

        def qt_writer(cb, g, accs):
            # qt[64*dsub + s, k, n] = Q[n, 2k+dsub, s] (already normalized)
            qt = qop.tile([128, 8, 128], BF16, tag="qo", name="qt")
            nc.vector.memset(qt[0:64, 0, :], 0.0)   # d=0 plane
            for (wi, so, do, w) in CB_SRC[cb]:
                acc_t = accs[wi]
                # dsub=1: d = 2k+1, k=0..7 -> planes 0,2,..,14
                nc.scalar.copy(qt[64:128, :, do:do + w],
                               acc_t[64 * g:64 * g + 64, 0:15:2, so:so + w])
                # dsub=0: d = 2k, k=1..7 -> planes 1,3,..,13
                nc.scalar.copy(qt[0:64, 1:8, do:do + w],
                               acc_t[64 * g:64 * g + 64, 1:14:2, so:so + w])
            cb_out = 8 * g + cb
            if cb_out == 15:
                nc.vector.tensor_mul(qt[:], qt[:], qmask[:])
            wp = wps.tile([128, 2, 512], F32, tag="wps", name="wp")
            for k in range(8):
                for h in range(2):
                    nc.tensor.matmul(wp[:, h, :], qt[:, k, :],
                                     ww_sb[:, k, 512 * h:512 * h + 512],
                                     start=(k == 0), stop=(k == 7))
            osb = qop.tile([128, 1024], BF16, tag="qo", name="osb")
            nc.scalar.copy(osb[:, 0:512], wp[:, 0, :])
            nc.scalar.copy(osb[:, 512:1024], wp[:, 1, :])
            nc.sync.dma_start(out_d[cb_out], osb[:])

        # ---------------- pipelined issue order
        xts = {}
        for i in range(4):
            xts[i] = load_x(i)
        for i in range(4):
            reader_supertile(i, xts[i])
        accs = {}
        tree_z(0, 256)
        accs[0] = mac_window(*SCS[0])
        for g in (0, 1):
            qt_writer(0, g, accs)
        tree_z(256, 256)
        accs[1] = mac_window(*SCS[1])
        for g in (0, 1):
            qt_writer(1, g, accs)
            qt_writer(2, g, accs)
        tree_z(512, 256)
        accs[2] = mac_window(*SCS[2])
        for g in (0, 1):
            qt_writer(3, g, accs)
            qt_writer(4, g, accs)
        tree_z(768, 256)
        accs[3] = mac_window(*SCS[3])
        for g in (0, 1):
            qt_writer(5, g, accs)
            qt_writer(6, g, accs)
        accs[4] = mac_window(*SCS[4])
        for g in (0, 1):
            qt_writer(7, g, accs)

    nc.compile()
    return nc


def _host_prep(x, W_r, b_r, W_w, b_w):
    """Build the 8 per-core input maps."""
    xf = np.asarray(x, np.float32).reshape(B * N, PD)
    W_r = np.asarray(W_r, np.float32)
    b_r = np.asarray(b_r, np.float32).reshape(-1)
    wr = W_r.astype(bfnp)
    ww = np.asarray(W_w, np.float32).astype(bfnp)
    br = np.ascontiguousarray(b_r.reshape(16, 128).T)
    wr_t = np.ascontiguousarray(wr.reshape(8, 128, 2048))
    ww_t = np.ascontiguousarray(ww.reshape(8, 128, 1024))

    in_maps = []
    for c in range(8):
        lo = c * ROWS_PER_CORE
        rows = xf[lo:lo + ROWS_PER_CORE].astype(bfnp)
        # supertile i: rows [256i, 256i+256) | [1024+256i, 1024+256i+256)
        # x4[i, ks, k, n] = rows[row(i, n), 128*ks + k]
        r2 = rows.reshape(2, 4, 256, 8, 128)          # [g, i, n', ks, k]
        x4 = np.ascontiguousarray(
            r2.transpose(1, 3, 4, 0, 2).reshape(4, 8, 128, 512))
        # host halo R: 16 rows after this core's chunk (zeros for odd cores)
        halo = np.zeros((16, PD), np.float32)
        if c % 2 == 0:
            halo = xf[lo + ROWS_PER_CORE: lo + ROWS_PER_CORE + 16]
        rhal = (halo @ W_r[:, ALL:] + b_r[ALL:]).reshape(16, 16, 64)
        rh = np.ascontiguousarray(
            rhal[:, :15, :].transpose(2, 1, 0)).astype(bfnp)  # [64, 15, 16]
        qmask = np.ones((128, 8, 128), np.float32)
        if c % 2 == 1:
            dsub = (np.arange(128)[:, None, None] // 64)
            k = np.arange(8)[None, :, None]
            n = np.arange(128)[None, None, :]
            bad = (n >= 112) & ((n - 112 + 2 * k + dsub) >= 16)
            qmask[np.broadcast_to(bad, (128, 8, 128))] = 0.0
        in_maps.append({
            "x4": x4, "rh": rh,
            "wr": wr_t, "ww": ww_t, "br": br,
            "qmask": qmask.astype(bfnp),
        })
    return in_maps


def kernel(x, W_r, b_r, W_w, b_w):
    if "nc" not in _cache:
        _cache["nc"] = _build()
    nc = _cache["nc"]
    in_maps = _host_prep(x, W_r, b_r, W_w, b_w)
    res = bass_utils.run_bass_kernel_spmd(nc, in_maps, core_ids=list(range(8)))
    out = np.concatenate([np.asarray(r["out"], dtype=np.float32)
                          .reshape(ROWS_PER_CORE, ALL)
                          for r in res.results], axis=0)
    out = out.reshape(B, N, ALL)
    out += np.asarray(b_w, np.float32).reshape(1, 1, ALL)
    return np.ascontiguousarray(out)


# revision 16
# speedup vs baseline: 1.0542x; 1.0542x over previous
"""TRN2 Bass kernel for nn_ChartOperator (sparse_attention), v4.

Math (B=4, N=4096, PD=1024, D=16, S=64, ALL=1024):
  P = x @ W_r + b_r
  L = P[..., :ALL].reshape(n, D, S); R = P[..., ALL:].reshape(n, D, S)
  e = exp(L); Z = sum_a e[n, a, s]
  Q[n, d, s] = (1/Z[n,s]) * sum_{a<d} e[n,a,s] * R[n+a+1, d-1-a, s]
  (last D rows of each batch: Q[t+d>=16] zeroed)
  out = Q.reshape(n, ALL) @ W_w + b_w

Sharding: 8 cores data-parallel over flattened (B*N) rows, 2048 rows/core.
The 16-row forward halo R is computed on the HOST (16x1024x1024 matmul per
core) and DMA'd straight into the rt tail; odd cores get bias-only halo
(their affected outputs are exactly the masked ones).

Two row-chains packed in partitions: chain g0 = rows 0..1023, g1 = rows
1024..2047, partition = (g, s). Each reader supertile covers 256 rows of
BOTH chains (rows [256i, 256i+256) u [1024+256i, ...)), so every supertile
unlocks a full 256-col MAC window across both chains and the DVE MAC
streams continuously behind the reader instead of stalling on the tail.

Engine budget (per core): PE = reader 112.6us + writer 54.6us (matmul only,
banded MAC is NOT on PE). DVE = products + adds (bf16 2x; misaligned bf16
still runs 2x, measured) + tree-Z + qt(rz-folded) ~ 165us. ACT = exp/ident
PSUM drains + osb. GpSimd: DMA issue only - concurrent GpSimd tensor ops
stall DVE on SBUF ports (measured: full op-duration stalls).
"""
import numpy as np
import ml_dtypes
from contextlib import ExitStack

import concourse.bass as bass
import concourse.tile as tile
from concourse import bacc, mybir
from concourse import bass_utils

BF16 = mybir.dt.bfloat16
F32 = mybir.dt.float32
bfnp = ml_dtypes.bfloat16
AF = mybir.ActivationFunctionType

B, N, PD = 4, 4096, 1024
D, S = 16, 64
ALL = D * S
ROWS_PER_CORE = 2048
RT_LEN = 1040                  # rt chain length (max read idx 1038)

# MAC windows (n0, W): read limit n0+W-1+15 must stay within the cols
# unlocked by supertiles 0..i (256*(i+1)-1) resp. 1039 for the last.
SCS = [(0, 240), (240, 256), (496, 256), (752, 256), (1008, 16)]
# cb -> list of (window idx, src_off, dst_off, width)
CB_SRC = {
    0: [(0, 0, 0, 128)],
    1: [(0, 128, 0, 112), (1, 0, 112, 16)],
    2: [(1, 16, 0, 128)],
    3: [(1, 144, 0, 112), (2, 0, 112, 16)],
    4: [(2, 16, 0, 128)],
    5: [(2, 144, 0, 112), (3, 0, 112, 16)],
    6: [(3, 16, 0, 128)],
    7: [(3, 144, 0, 112), (4, 0, 112, 16)],
}

_cache = {}


def _build(debug=False):
    nc = bacc.Bacc("TRN2", target_bir_lowering=False, debug=False, num_devices=8)

    x4_d = nc.dram_tensor("x4", [4, 8, 128, 512], BF16, kind="ExternalInput").ap()
    rh_d = nc.dram_tensor("rh", [64, 15, 16], BF16, kind="ExternalInput").ap()
    wr_d = nc.dram_tensor("wr", [8, 128, 2048], BF16, kind="ExternalInput").ap()
    ww_d = nc.dram_tensor("ww", [8, 128, 1024], BF16, kind="ExternalInput").ap()
    br_d = nc.dram_tensor("br", [128, 16], F32, kind="ExternalInput").ap()
    qmask_d = nc.dram_tensor("qmask", [128, 8, 128], BF16, kind="ExternalInput").ap()
    out_d = nc.dram_tensor("out", [16, 128, 1024], BF16, kind="ExternalOutput").ap()

    with tile.TileContext(nc) as tc, ExitStack() as ctx, \
            nc.allow_low_precision(
                reason="bf16 MAC/tree validated vs reference: rel 7.3e-3 "
                       "< 2e-2 tolerance (numpy bit-model)"):
        cpool = ctx.enter_context(tc.tile_pool(name="cpool", bufs=1))
        xpool = ctx.enter_context(tc.tile_pool(name="xpool", bufs=3))
        treep = ctx.enter_context(tc.tile_pool(name="treep", bufs=2))
        prodp = ctx.enter_context(tc.tile_pool(name="prodp", bufs=3))
        accp = ctx.enter_context(tc.tile_pool(name="accp", bufs=2))
        qop = ctx.enter_context(tc.tile_pool(name="qop", bufs=4))
        psp = ctx.enter_context(tc.tile_pool(name="psp", bufs=3, space="PSUM"))
        wps = ctx.enter_context(tc.tile_pool(name="wps", bufs=2, space="PSUM"))

        # --- persistent constants / big buffers
        wr_sb = cpool.tile([128, 8, 2048], BF16)
        ww_sb = cpool.tile([128, 8, 1024], BF16)
        br_sb = cpool.tile([128, 16], F32)
        qmask = cpool.tile([128, 8, 128], BF16)
        et = cpool.tile([128, 16, 1024], BF16)    # [(g2,s64), a, n-chain] raw e
        rt = cpool.tile([128, 15, RT_LEN], BF16)  # [(g2,s64), c, n-chain] R
        rz = cpool.tile([128, 1024], BF16)        # 1/Z per chain col

        # weights on sync queue (first ks-pair first)
        for i in range(4):
            nc.sync.dma_start(wr_sb[:, 2 * i:2 * i + 2, :],
                              wr_d[2 * i:2 * i + 2].rearrange("k p c -> p k c"))
        nc.gpsimd.dma_start(br_sb[:], br_d[:])
        for i in range(2):
            nc.sync.dma_start(ww_sb[:, 4 * i:4 * i + 4, :],
                              ww_d[4 * i:4 * i + 4].rearrange("k p c -> p k c"))
        nc.gpsimd.dma_start(qmask[:], qmask_d[:])
        # host-computed halo R -> rt g1 tail
        nc.gpsimd.dma_start(rt[64:128, :, 1024:1040], rh_d[:])

        def load_x(i):
            xt = xpool.tile([128, 8, 512], BF16, tag="xk", name=f"x{i}")
            nc.gpsimd.dma_start(xt[:], x4_d[i].rearrange("k p n -> p k n"))
            return xt

        def reader_supertile(i, xt):
            # supertile i: rows [256i, 256i+256) = g0 cols [256i..] in ps
            # cols 0-255, rows [1024+256i, ..) = g1 cols in ps cols 256-511.
            n0 = 256 * i
            for u in range(16):           # col slabs: 0-7 = L, 8-15 = R
                is_l = u < 8
                ps = psp.tile([128, 512], F32, tag="ps", name="ps")
                for ks in range(8):
                    nc.tensor.matmul(ps[:], wr_sb[:, ks, 128 * u:128 * (u + 1)],
                                     xt[:, ks, :],
                                     start=(ks == 0), stop=(ks == 7))
                for dsub in range(2):
                    bias = br_sb[64 * dsub:64 * dsub + 64, u:u + 1]
                    for g in range(2):
                        src = ps[64 * dsub:64 * dsub + 64,
                                 256 * g:256 * g + 256]
                        if is_l:
                            a = 2 * u + dsub
                            nc.scalar.activation(
                                et[64 * g:64 * g + 64, a, n0:n0 + 256], src,
                                AF.Exp, bias=bias)
                        else:
                            c = 2 * (u - 8) + dsub
                            if c == 15:
                                continue  # c=15 never read by the MAC
                            nc.scalar.activation(
                                rt[64 * g:64 * g + 64, c, n0:n0 + 256], src,
                                AF.Identity, bias=bias)
                            if i == 0 and g == 1:
                                # rows 1024-1039 are ALSO g0 chain cols 1024+
                                nc.scalar.activation(
                                    rt[0:64, c, 1024:1040],
                                    ps[64 * dsub:64 * dsub + 64, 256:272],
                                    AF.Identity, bias=bias)

        def tree_z(w0, W):
            # rz = 1 / sum_a e over the 16 a-planes for chain cols [w0, w0+W)
            t1 = treep.tile([128, 8, 256], BF16, tag="tree", name="t1")
            nc.vector.tensor_add(t1[:, :, :W], et[:, 0:16:2, w0:w0 + W],
                                 et[:, 1:16:2, w0:w0 + W])
            t2 = treep.tile([128, 4, 256], BF16, tag="tree", name="t2")
            nc.vector.tensor_add(t2[:, :, :W], t1[:, 0:8:2, :W], t1[:, 1:8:2, :W])
            t3 = treep.tile([128, 2, 256], BF16, tag="tree", name="t3")
            nc.vector.tensor_add(t3[:, :, :W], t2[:, 0:4:2, :W], t2[:, 1:4:2, :W])
            zf = treep.tile([128, 256], F32, tag="tree", name="zf")
            nc.vector.tensor_add(zf[:, :W], t3[:, 0, :W], t3[:, 1, :W])
            nc.vector.reciprocal(rz[:, w0:w0 + W], zf[:, :W])

        def mac_window(n0, W):
            # acc plane i (=d-1) over cols [n0, n0+W):
            #   acc[:, a+c, :] += e[:, a, n] * rt[:, c, n+a+1]
            # descending a: plane a is init'ed by its c=0 term. All on DVE:
            # concurrent GpSimd tensor ops stall DVE on SBUF ports, and
            # misaligned bf16 still runs 2x on DVE.
            acc = accp.tile([128, 15, 256], BF16, tag="acc", name="acc")
            for a in range(14, -1, -1):
                cnt = 15 - a
                sh = a + 1
                def eb(k):
                    return et[:, a:a + 1, n0:n0 + W].to_broadcast((128, k, W))
                nc.vector.tensor_mul(acc[:, a:a + 1, 0:W], eb(1),
                                     rt[:, 0:1, n0 + sh:n0 + sh + W])
                if cnt > 1:
                    k = cnt - 1
                    p = prodp.tile([128, 14, 256], BF16, tag="p", name="p")
                    nc.vector.tensor_mul(p[:, 0:k, 0:W], eb(k),
                                         rt[:, 1:cnt, n0 + sh:n0 + sh + W])
                    nc.vector.tensor_add(acc[:, a + 1:a + 1 + k, 0:W],
                                         acc[:, a + 1:a + 1 + k, 0:W],
                                         p[:, 0:k, 0:W])
            return acc

        def qt_writer(cb, g, accs):
            # qt[64*dsub + s, k, n] = Q[n, 2k+dsub, s] * rz
            col0 = 128 * cb
            qt = qop.tile([128, 8, 128], BF16, tag="qo", name="qt")
            nc.vector.memset(qt[0:64, 0, :], 0.0)   # d=0 plane
            for (wi, so, do, w) in CB_SRC[cb]:
                acc_t = accs[wi]
                rzs = rz[64 * g:64 * g + 64, col0 + do:col0 + do + w]
                rz8 = rzs.rearrange("p (o n) -> p o n", o=1).to_broadcast((64, 8, w))
                rz7 = rzs.rearrange("p (o n) -> p o n", o=1).to_broadcast((64, 7, w))
                # dsub=1: d = 2k+1, k=0..7 -> planes 0,2,..,14
                nc.vector.tensor_mul(qt[64:128, :, do:do + w],
                                     acc_t[64 * g:64 * g + 64, 0:15:2, so:so + w],
                                     rz8)
                # dsub=0: d = 2k, k=1..7 -> planes 1,3,..,13
                nc.vector.tensor_mul(qt[0:64, 1:8, do:do + w],
                                     acc_t[64 * g:64 * g + 64, 1:14:2, so:so + w],
                                     rz7)
            cb_out = 8 * g + cb
            if cb_out == 15:
                nc.vector.tensor_mul(qt[:], qt[:], qmask[:])
            wp = wps.tile([128, 2, 512], F32, tag="wps", name="wp")
            for k in range(8):
                for h in range(2):
                    nc.tensor.matmul(wp[:, h, :], qt[:, k, :],
                                     ww_sb[:, k, 512 * h:512 * h + 512],
                                     start=(k == 0), stop=(k == 7))
            osb = qop.tile([128, 1024], BF16, tag="qo", name="osb")
            nc.scalar.copy(osb[:, 0:512], wp[:, 0, :])
            nc.vector.tensor_copy(osb[:, 512:1024], wp[:, 1, :])
            nc.sync.dma_start(out_d[cb_out], osb[:])

        # ---------------- pipelined issue order
        xts = {}
        for i in range(4):
            xts[i] = load_x(i)
        for i in range(4):
            reader_supertile(i, xts[i])
        accs = {}
        tree_z(0, 256)
        accs[0] = mac_window(*SCS[0])
        for g in (0, 1):
            qt_writer(0, g, accs)
        tree_z(256, 256)
        accs[1] = mac_window(*SCS[1])
        for g in (0, 1):
            qt_writer(1, g, accs)
            qt_writer(2, g, accs)
        tree_z(512, 256)
        accs[2] = mac_window(*SCS[2])
        for g in (0, 1):
            qt_writer(3, g, accs)
            qt_writer(4, g, accs)
        tree_z(768, 256)
        accs[3] = mac_window(*SCS[3])
        for g in (0, 1):
            qt_writer(5, g, accs)
            qt_writer(6, g, accs)
        accs[4] = mac_window(*SCS[4])
        for g in (0, 1):
            qt_writer(7, g, accs)

    nc.compile()
    return nc


def _host_prep(x, W_r, b_r, W_w, b_w):
    """Build the 8 per-core input maps."""
    xf = np.asarray(x, np.float32).reshape(B * N, PD)
    W_r = np.asarray(W_r, np.float32)
    b_r = np.asarray(b_r, np.float32).reshape(-1)
    wr = W_r.astype(bfnp)
    ww = np.asarray(W_w, np.float32).astype(bfnp)
    br = np.ascontiguousarray(b_r.reshape(16, 128).T)
    wr_t = np.ascontiguousarray(wr.reshape(8, 128, 2048))
    ww_t = np.ascontiguousarray(ww.reshape(8, 128, 1024))

    in_maps = []
    for c in range(8):
        lo = c * ROWS_PER_CORE
        rows = xf[lo:lo + ROWS_PER_CORE].astype(bfnp)
        # supertile i: rows [256i, 256i+256) | [1024+256i, 1024+256i+256)
        # x4[i, ks, k, n] = rows[row(i, n), 128*ks + k]
        r2 = rows.reshape(2, 4, 256, 8, 128)          # [g, i, n', ks, k]
        x4 = np.ascontiguousarray(
            r2.transpose(1, 3, 4, 0, 2).reshape(4, 8, 128, 512))
        # host halo R: 16 rows after this core's chunk (zeros for odd cores)
        halo = np.zeros((16, PD), np.float32)
        if c % 2 == 0:
            halo = xf[lo + ROWS_PER_CORE: lo + ROWS_PER_CORE + 16]
        rhal = (halo @ W_r[:, ALL:] + b_r[ALL:]).reshape(16, 16, 64)
        rh = np.ascontiguousarray(
            rhal[:, :15, :].transpose(2, 1, 0)).astype(bfnp)  # [64, 15, 16]
        qmask = np.ones((128, 8, 128), np.float32)
        if c % 2 == 1:
            dsub = (np.arange(128)[:, None, None] // 64)
            k = np.arange(8)[None, :, None]
            n = np.arange(128)[None, None, :]
            bad = (n >= 112) & ((n - 112 + 2 * k + dsub) >= 16)
            qmask[np.broadcast_to(bad, (128, 8, 128))] = 0.0
        in_maps.append({
            "x4": x4, "rh": rh,
            "wr": wr_t, "ww": ww_t, "br": br,
            "qmask": qmask.astype(bfnp),
        })
    return in_maps


def kernel(x, W_r, b_r, W_w, b_w):
    if "nc" not in _cache:
        _cache["nc"] = _build()
    nc = _cache["nc"]
    in_maps = _host_prep(x, W_r, b_r, W_w, b_w)
    res = bass_utils.run_bass_kernel_spmd(nc, in_maps, core_ids=list(range(8)))
    out = np.concatenate([np.asarray(r["out"], dtype=np.float32)
                          .reshape(ROWS_PER_CORE, ALL)
                          for r in res.results], axis=0)
    out = out.reshape(B, N, ALL)
    out += np.asarray(b_w, np.float32).reshape(1, 1, ALL)
    return np.ascontiguousarray(out)
